# revision 28
# baseline (speedup 1.0000x reference)
"""Confusion-matrix (joint histogram) kernel for Trainium2.

Math: out[b, i, j] = #{pixels p in batch b : yp[b,p] == i and y[b,p] == j}
for i, j in [0, 21). Inputs yp, y are [8, 2048, 2048] int32, values in [0, 21).

Per NeuronCore (core c processes batch c):
  - DMA int32 pixel chunks into SBUF; ScalarE converts to bf16,
  - feature planes in matmul-ready interleaved layout
    (planes[p, blk*BW + u*6 + g]), split across engines by feature kind:
      u < D:  delta planes  (x == u)        via DVE tensor_scalar(is_equal), 4x
      u == D: constant ones plane           via one-time memset (pool bufs
              are reused round-robin, so the ones/pad columns persist)
      u > D:  sign-step planes sign(x-u+.5) via ScalarE activation(Sign), +-1
    The 21 features {delta_0..delta_{D-1}, 1, sigma_{D+1}..sigma_20} span the
    one-hot basis, so host-side inversion recovers exact counts.
  - joint counts via TensorE: planes(yp)^T @ planes(y), 6 pixel-columns per
    matmul ([128, BW] x [128, 126]) accumulated into one PSUM [128, 126]
    f32 tile (exact integers < 2^24),
  - host: sum 6 diagonal 21x21 blocks (g::6), invert the basis transform,
    and histogram the (<1%) tail columns that skip the device entirely.
"""

import numpy as np


def _ensure_axon_hooks_stub():
    """bass_utils imports antenv.axon_hooks when tracing is requested (e.g.
    via BASS_TRACE=1 in the environment). Some images lack that module; a
    stub returning no hook makes bass_utils degrade gracefully (skip trace,
    still run) instead of crashing. Never shadows a real module."""
    try:
        import antenv.axon_hooks  # noqa: F401
    except ImportError:
        import sys
        import types

        try:
            import antenv
        except ImportError:
            return
        mod = types.ModuleType("antenv.axon_hooks")
        mod._hook = None
        mod.set_axon_ntff_profile_hook = lambda h: setattr(mod, "_hook", h)
        mod.get_axon_ntff_profile_hook = lambda: mod._hook
        sys.modules["antenv.axon_hooks"] = mod
        antenv.axon_hooks = mod


_ensure_axon_hooks_stub()

C = 21                  # classes
G = 6                   # pixel-column groups per matmul (G*C = 126 <= 128)
P = 128                 # partitions
BW = 128                # block width: 126 used + 2 pad (128 => FWL weight load)
FP = 756                # pixel-columns per tensor per chunk (divisible by 6)
N_ACT = 4               # top classes as sign-step planes on ScalarE

_CACHE = {}


def _build(
    n_free,
    fp=FP,
    bw=BW,
    n_act=N_ACT,
    work_cols=None,
    repeat=1,
    skip_mm=False,
    cat_bufs=3,
    plane_bufs=2,
):
    import concourse.bacc as bacc
    import concourse.mybir as mybir
    import concourse.tile as tile
    from contextlib import nullcontext

    if work_cols is None:
        work_cols = n_free

    nc = bacc.Bacc(
        "TRN2",
        target_bir_lowering=False,
        debug=False,
        enable_asserts=False,
        num_devices=8,
    )
    yp = nc.dram_tensor("yp", [P, n_free], mybir.dt.int32, kind="ExternalInput").ap()
    y = nc.dram_tensor("y", [P, n_free], mybir.dt.int32, kind="ExternalInput").ap()
    out = nc.dram_tensor("out", [P, 126], mybir.dt.float32, kind="ExternalOutput").ap()

    # Only full chunks run on-device; the remainder (< fp cols, <1% of
    # pixels) is histogrammed on the host with np.bincount. The last full
    # chunk is split small-tapered so the final matmul drain is short.
    n_main = (work_cols // fp) * fp
    chunk_ws = [fp] * (n_main // fp)
    if chunk_ws and fp >= 432:
        chunk_ws[-1:] = [fp - 216, 216]   # short final chunk: quick drain
        chunk_ws[:1] = [216, fp - 216]    # short first chunk: quick fill
    total_mms = n_main // G

    bf16 = mybir.dt.bfloat16
    f32 = mybir.dt.float32
    i32 = mybir.dt.int32
    Copy = mybir.ActivationFunctionType.Copy
    Sign = mybir.ActivationFunctionType.Sign
    n_dve = C - 1 - n_act        # ones plane sits at u = n_dve

    with tile.TileContext(nc) as tc:
        with (
            tc.tile_pool(name="psum", bufs=1, space="PSUM") as psum_pool,
            tc.tile_pool(name="cat", bufs=cat_bufs) as cat_pool,
            tc.tile_pool(name="planes", bufs=plane_bufs) as plane_pool,
            tc.tile_pool(name="singles", bufs=1) as singles,
        ):
            acc = psum_pool.tile([P, 126], f32)
            if n_act:
                bias_t = singles.tile([P, n_act], f32)
                for k in range(n_act):
                    nc.vector.memset(bias_t[:, k : k + 1], 0.5 - (n_dve + 1 + k))
                # dummy activation: trigger the ACT table load during the
                # preamble/first-DMA window instead of on the critical path
                warm = singles.tile([P, 1], bf16)
                nc.scalar.activation(warm[:], bias_t[:, 0:1], Sign)
            mm = 0
            chunk_idx = 0
            rep_ctx = tc.For_i(0, repeat, 1) if repeat > 1 else nullcontext()

            with rep_ctx:

                def do_plane_chunk(c16, wp, o, w):
                    """c16: [128, 2*wp] bf16 = [yp pair-cols | y pair-cols].
                    Process the chunk at column offset o, width w (w % 6 == 0).

                    planes[p, blk*BW + u*6 + g] = feat_u(vals[p, blk*6+g]),
                    blk in [0, 2*w/6). A-side = blks [0, w/6), B-side = rest.
                    Each matmul reads a contiguous [128, BW] slice.
                    """
                    nonlocal mm, chunk_idx
                    nblk = 2 * w // G
                    catv = (
                        c16.rearrange("p (s f) -> p s f", s=2)[:, :, o : o + w]
                        .rearrange("p s (b f) -> p s b f", f=G)
                    )
                    planes = plane_pool.tile([P, nblk_max * BW], bf16, tag="planes")
                    pl3 = planes[:, : nblk * BW].rearrange("p (b f) -> p b f", f=BW)
                    pl4 = planes[:, : nblk * BW].rearrange(
                        "p (s b f) -> p s b f", s=2, f=BW
                    )
                    if chunk_idx < plane_bufs:
                        # constant columns, written once per pool buffer: the
                        # ones plane (u = n_dve) and, for bw=128, the 2 pad
                        # columns. Pool bufs rotate round-robin so these
                        # persist; later chunks never touch them. Memset the
                        # full-size view so every chunk size is covered.
                        plf = planes[:, :].rearrange("p (b f) -> p b f", f=BW)
                        nc.vector.memset(
                            plf[:, :, n_dve * G : (n_dve + 1) * G], 1.0
                        )
                        if bw == 128:
                            nc.vector.memset(plf[:, :, 126:128], 0.0)
                    chunk_idx += 1
                    for u in range(n_dve):
                        nc.vector.tensor_scalar(
                            pl4[:, :, :, u * G : (u + 1) * G],
                            catv[:],
                            float(u),
                            None,
                            mybir.AluOpType.is_equal,
                        )
                    for u in range(n_dve + 1, C):
                        k = u - n_dve - 1
                        nc.scalar.activation(
                            pl4[:, :, :, u * G : (u + 1) * G],
                            catv[:],
                            Sign,
                            bias=bias_t[:, k : k + 1],
                        )
                    half = w // G
                    for t in (range(0) if skip_mm else range(w // G)):
                        nc.tensor.matmul(
                            acc[:, :],
                            pl3[:, t, :bw].rearrange("p f -> p f"),
                            pl3[:, half + t, :126].rearrange("p f -> p f"),
                            start=(mm == 0),
                            stop=(mm == total_mms - 1),
                        )
                        mm += 1

                nblk_max = 2 * fp // G

                # chunks are processed in pairs sharing one DMA + one convert
                pairs = [
                    tuple(chunk_ws[i : i + 2]) for i in range(0, len(chunk_ws), 2)
                ]
                off = 0
                for pw in pairs:
                    wp = sum(pw)
                    cat32 = cat_pool.tile([P, 4 * fp], i32, tag="cat32")
                    nc.sync.dma_start(cat32[:, :wp], yp[:, off : off + wp])
                    nc.sync.dma_start(
                        cat32[:, wp : 2 * wp], y[:, off : off + wp]
                    )
                    cat16 = cat_pool.tile([P, 4 * fp], bf16, tag="cat16")
                    nc.scalar.activation(
                        cat16[:, : 2 * wp], cat32[:, : 2 * wp], Copy
                    )
                    o = 0
                    for w in pw:
                        do_plane_chunk(cat16[:, : 2 * wp], wp, o, w)
                        o += w
                    off += wp

            assert skip_mm or mm == total_mms
            res = singles.tile([P, 126], f32)
            if skip_mm:
                nc.vector.memset(res[:], 0.0)
            else:
                nc.vector.tensor_copy(res[:], acc[:, :])
            nc.sync.dma_start(out, res[:])

    nc.compile()
    return nc


def _feature_eval(n_act, x):
    """Feature vector phi(x): deltas, ones, sign-steps. Works for any x
    (including the sentinel)."""
    n_dve = C - 1 - n_act
    v = np.zeros(C, dtype=np.float64)
    for u in range(n_dve):
        v[u] = 1.0 if x == u else 0.0
    v[n_dve] = 1.0
    for u in range(n_dve + 1, C):
        v[u] = 1.0 if x >= u else -1.0
    return v


def _basis_matrix(n_act):
    """Phi[u, x] = feature u evaluated at class value x."""
    return np.stack([_feature_eval(n_act, x) for x in range(C)], axis=1)


def _get(n_free):
    if n_free not in _CACHE:
        _CACHE[n_free] = _build(n_free)
    return _CACHE[n_free]


def kernel(yp, y, res, n_classes, _trace=False):
    from concourse import bass_utils

    yp = np.ascontiguousarray(np.asarray(yp))
    y = np.ascontiguousarray(np.asarray(y))
    B = yp.shape[0]
    n_free = yp[0].size // P
    nc = _get(n_free)
    in_maps = [
        {"yp": yp[b].reshape(P, n_free), "y": y[b].reshape(P, n_free)}
        for b in range(B)
    ]
    r = bass_utils.run_bass_kernel_spmd(
        nc, in_maps, core_ids=list(range(B)), trace=_trace
    )

    # Host-side reconstruction
    n_act = N_ACT
    n_main = (n_free // FP) * FP
    phi = _basis_matrix(n_act)
    phi_inv = np.linalg.inv(phi)

    outs = []
    for b in range(B):
        Pm = r.results[b]["out"].astype(np.float64)
        M = np.zeros((C, C), np.float64)
        for g in range(G):
            M += Pm[g:126:G, g:126:G]
        Cb = phi_inv @ M @ phi_inv.T
        if n_main < n_free:  # tail pixels histogrammed on host
            ypt = yp[b].reshape(P, n_free)[:, n_main:].ravel()
            yt = y[b].reshape(P, n_free)[:, n_main:].ravel()
            Cb = Cb + np.bincount(
                ypt * C + yt, minlength=C * C
            ).reshape(C, C)
        outs.append(Cb)
    res_np = np.stack(outs).astype(np.float32)
    if _trace:
        kernel._last_results = r
    return res_np


# revision 29
# speedup vs baseline: 1.0209x; 1.0209x over previous
"""Confusion-matrix (joint histogram) kernel for Trainium2.

Math: out[b, i, j] = #{pixels p in batch b : yp[b,p] == i and y[b,p] == j}
for i, j in [0, 21). Inputs yp, y are [8, 2048, 2048] int32, values in [0, 21).

Per NeuronCore (core c processes batch c):
  - DMA int32 pixel chunks into SBUF; ScalarE converts to bf16,
  - feature planes in matmul-ready interleaved layout
    (planes[p, blk*BW + u*6 + g]), split across engines by feature kind:
      u < D:  delta planes  (x == u)        via DVE tensor_scalar(is_equal), 4x
      u == D: constant ones plane           via one-time memset (pool bufs
              are reused round-robin, so the ones/pad columns persist)
      u > D:  sign-step planes sign(x-u+.5) via ScalarE activation(Sign), +-1
    The 21 features {delta_0..delta_{D-1}, 1, sigma_{D+1}..sigma_20} span the
    one-hot basis, so host-side inversion recovers exact counts.
  - joint counts via TensorE: planes(yp)^T @ planes(y), 6 pixel-columns per
    matmul ([128, BW] x [128, 126]) accumulated into one PSUM [128, 126]
    f32 tile (exact integers < 2^24),
  - host: sum 6 diagonal 21x21 blocks (g::6), invert the basis transform,
    and histogram the (<1%) tail columns that skip the device entirely.
"""

import numpy as np


def _ensure_axon_hooks_stub():
    """bass_utils imports antenv.axon_hooks when tracing is requested (e.g.
    via BASS_TRACE=1 in the environment). Some images lack that module; a
    stub returning no hook makes bass_utils degrade gracefully (skip trace,
    still run) instead of crashing. Never shadows a real module."""
    try:
        import antenv.axon_hooks  # noqa: F401
    except ImportError:
        import sys
        import types

        try:
            import antenv
        except ImportError:
            return
        mod = types.ModuleType("antenv.axon_hooks")
        mod._hook = None
        mod.set_axon_ntff_profile_hook = lambda h: setattr(mod, "_hook", h)
        mod.get_axon_ntff_profile_hook = lambda: mod._hook
        sys.modules["antenv.axon_hooks"] = mod
        antenv.axon_hooks = mod


_ensure_axon_hooks_stub()

C = 21                  # classes
G = 6                   # pixel-column groups per matmul (G*C = 126 <= 128)
P = 128                 # partitions
BW = 128                # block width: 126 used + 2 pad (128 => FWL weight load)
FP = 756                # pixel-columns per tensor per chunk (divisible by 6)
N_ACT = 4               # top classes as sign-step planes on ScalarE

_CACHE = {}


def _build(
    n_free,
    fp=FP,
    bw=BW,
    n_act=N_ACT,
    work_cols=None,
    repeat=1,
    skip_mm=False,
    cat_bufs=3,
    plane_bufs=2,
):
    import concourse.bacc as bacc
    import concourse.mybir as mybir
    import concourse.tile as tile
    from contextlib import nullcontext

    if work_cols is None:
        work_cols = n_free

    nc = bacc.Bacc(
        "TRN2",
        target_bir_lowering=False,
        debug=False,
        enable_asserts=False,
        num_devices=8,
    )
    yp = nc.dram_tensor("yp", [P, n_free], mybir.dt.int32, kind="ExternalInput").ap()
    y = nc.dram_tensor("y", [P, n_free], mybir.dt.int32, kind="ExternalInput").ap()
    out = nc.dram_tensor("out", [P, 126], mybir.dt.float32, kind="ExternalOutput").ap()

    # Only full chunks run on-device; the remainder (< fp cols, <1% of
    # pixels) is histogrammed on the host with np.bincount. The last full
    # chunk is split small-tapered so the final matmul drain is short.
    n_main = (work_cols // fp) * fp
    chunk_ws = [fp] * (n_main // fp)
    if chunk_ws and fp >= 432:
        chunk_ws[-1:] = [fp - 216, 216]   # short final chunk: quick drain
        chunk_ws[:1] = [216, fp - 216]    # short first chunk: quick fill
    total_mms = n_main // G

    bf16 = mybir.dt.bfloat16
    f32 = mybir.dt.float32
    i32 = mybir.dt.int32
    Copy = mybir.ActivationFunctionType.Copy
    Sign = mybir.ActivationFunctionType.Sign
    n_dve = C - 1 - n_act        # ones plane sits at u = n_dve

    with tile.TileContext(nc) as tc:
        with (
            tc.tile_pool(name="psum", bufs=1, space="PSUM") as psum_pool,
            tc.tile_pool(name="cat", bufs=cat_bufs) as cat_pool,
            tc.tile_pool(name="planes", bufs=plane_bufs) as plane_pool,
            tc.tile_pool(name="singles", bufs=1) as singles,
        ):
            acc = psum_pool.tile([P, 126], f32)
            if n_act:
                bias_t = singles.tile([P, n_act], f32)
                for k in range(n_act):
                    nc.vector.memset(bias_t[:, k : k + 1], 0.5 - (n_dve + 1 + k))
                # dummy activation: trigger the ACT table load during the
                # preamble/first-DMA window instead of on the critical path
                warm = singles.tile([P, 1], bf16)
                nc.scalar.activation(warm[:], bias_t[:, 0:1], Sign)
            mm = 0
            chunk_idx = 0
            rep_ctx = tc.For_i(0, repeat, 1) if repeat > 1 else nullcontext()

            with rep_ctx:

                def do_plane_chunk(c16, wp, o, w):
                    """c16: [128, 2*wp] bf16 = [yp pair-cols | y pair-cols].
                    Process the chunk at column offset o, width w (w % 6 == 0).

                    planes[p, blk*BW + u*6 + g] = feat_u(vals[p, blk*6+g]),
                    blk in [0, 2*w/6). A-side = blks [0, w/6), B-side = rest.
                    Each matmul reads a contiguous [128, BW] slice.
                    """
                    nonlocal mm, chunk_idx
                    nblk = 2 * w // G
                    catv = (
                        c16.rearrange("p (s f) -> p s f", s=2)[:, :, o : o + w]
                        .rearrange("p s (b f) -> p s b f", f=G)
                    )
                    planes = plane_pool.tile([P, nblk_max * BW], bf16, tag="planes")
                    pl3 = planes[:, : nblk * BW].rearrange("p (b f) -> p b f", f=BW)
                    pl4 = planes[:, : nblk * BW].rearrange(
                        "p (s b f) -> p s b f", s=2, f=BW
                    )
                    if chunk_idx < plane_bufs:
                        # constant columns, written once per pool buffer: the
                        # ones plane (u = n_dve) and, for bw=128, the 2 pad
                        # columns. Pool bufs rotate round-robin so these
                        # persist; later chunks never touch them. Memset the
                        # full-size view so every chunk size is covered.
                        plf = planes[:, :].rearrange("p (b f) -> p b f", f=BW)
                        nc.vector.memset(
                            plf[:, :, n_dve * G : (n_dve + 1) * G], 1.0
                        )
                        if bw == 128:
                            nc.vector.memset(plf[:, :, 126:128], 0.0)
                    chunk_idx += 1
                    for u in range(n_dve):
                        nc.vector.tensor_scalar(
                            pl4[:, :, :, u * G : (u + 1) * G],
                            catv[:],
                            float(u),
                            None,
                            mybir.AluOpType.is_equal,
                        )
                    for u in range(n_dve + 1, C):
                        k = u - n_dve - 1
                        nc.scalar.activation(
                            pl4[:, :, :, u * G : (u + 1) * G],
                            catv[:],
                            Sign,
                            bias=bias_t[:, k : k + 1],
                        )
                    half = w // G
                    for t in (range(0) if skip_mm else range(w // G)):
                        nc.tensor.matmul(
                            acc[:, :],
                            pl3[:, t, :bw].rearrange("p f -> p f"),
                            pl3[:, half + t, :126].rearrange("p f -> p f"),
                            start=(mm == 0),
                            stop=(mm == total_mms - 1),
                        )
                        mm += 1

                nblk_max = 2 * fp // G

                # chunks are processed in pairs sharing one DMA + one convert;
                # the small first chunk goes solo so the pipeline primes fast
                pairs = [tuple(chunk_ws[:1])] + [
                    tuple(chunk_ws[i : i + 2]) for i in range(1, len(chunk_ws), 2)
                ]
                off = 0
                for pw in pairs:
                    wp = sum(pw)
                    cat32 = cat_pool.tile([P, 4 * fp], i32, tag="cat32")
                    nc.sync.dma_start(cat32[:, :wp], yp[:, off : off + wp])
                    nc.sync.dma_start(
                        cat32[:, wp : 2 * wp], y[:, off : off + wp]
                    )
                    cat16 = cat_pool.tile([P, 4 * fp], bf16, tag="cat16")
                    nc.scalar.activation(
                        cat16[:, : 2 * wp], cat32[:, : 2 * wp], Copy
                    )
                    o = 0
                    for w in pw:
                        do_plane_chunk(cat16[:, : 2 * wp], wp, o, w)
                        o += w
                    off += wp

            assert skip_mm or mm == total_mms
            res = singles.tile([P, 126], f32)
            if skip_mm:
                nc.vector.memset(res[:], 0.0)
            else:
                nc.vector.tensor_copy(res[:], acc[:, :])
            nc.sync.dma_start(out, res[:])

    nc.compile()
    return nc


def _feature_eval(n_act, x):
    """Feature vector phi(x): deltas, ones, sign-steps. Works for any x
    (including the sentinel)."""
    n_dve = C - 1 - n_act
    v = np.zeros(C, dtype=np.float64)
    for u in range(n_dve):
        v[u] = 1.0 if x == u else 0.0
    v[n_dve] = 1.0
    for u in range(n_dve + 1, C):
        v[u] = 1.0 if x >= u else -1.0
    return v


def _basis_matrix(n_act):
    """Phi[u, x] = feature u evaluated at class value x."""
    return np.stack([_feature_eval(n_act, x) for x in range(C)], axis=1)


def _get(n_free):
    if n_free not in _CACHE:
        _CACHE[n_free] = _build(n_free)
    return _CACHE[n_free]


def kernel(yp, y, res, n_classes, _trace=False):
    from concourse import bass_utils

    yp = np.ascontiguousarray(np.asarray(yp))
    y = np.ascontiguousarray(np.asarray(y))
    B = yp.shape[0]
    n_free = yp[0].size // P
    nc = _get(n_free)
    in_maps = [
        {"yp": yp[b].reshape(P, n_free), "y": y[b].reshape(P, n_free)}
        for b in range(B)
    ]
    r = bass_utils.run_bass_kernel_spmd(
        nc, in_maps, core_ids=list(range(B)), trace=_trace
    )

    # Host-side reconstruction
    n_act = N_ACT
    n_main = (n_free // FP) * FP
    phi = _basis_matrix(n_act)
    phi_inv = np.linalg.inv(phi)

    outs = []
    for b in range(B):
        Pm = r.results[b]["out"].astype(np.float64)
        M = np.zeros((C, C), np.float64)
        for g in range(G):
            M += Pm[g:126:G, g:126:G]
        Cb = phi_inv @ M @ phi_inv.T
        if n_main < n_free:  # tail pixels histogrammed on host
            ypt = yp[b].reshape(P, n_free)[:, n_main:].ravel()
            yt = y[b].reshape(P, n_free)[:, n_main:].ravel()
            Cb = Cb + np.bincount(
                ypt * C + yt, minlength=C * C
            ).reshape(C, C)
        outs.append(Cb)
    res_np = np.stack(outs).astype(np.float32)
    if _trace:
        kernel._last_results = r
    return res_np


# revision 31
# speedup vs baseline: 1.0513x; 1.0298x over previous
"""Confusion-matrix (joint histogram) kernel for Trainium2.

Math: out[b, i, j] = #{pixels p in batch b : yp[b,p] == i and y[b,p] == j}
for i, j in [0, 21). Inputs yp, y are [8, 2048, 2048] int32, values in [0, 21).

Per NeuronCore (core c processes batch c):
  - DMA int32 pixel chunks into SBUF; ScalarE converts to bf16,
  - feature planes in matmul-ready interleaved layout
    (planes[p, blk*BW + u*6 + g]), split across engines by feature kind:
      u < D:  delta planes  (x == u)        via DVE tensor_scalar(is_equal), 4x
      u == D: constant ones plane           via one-time memset (pool bufs
              are reused round-robin, so the ones/pad columns persist)
      u > D:  sign-step planes sign(x-u+.5) via ScalarE activation(Sign), +-1
    The 21 features {delta_0..delta_{D-1}, 1, sigma_{D+1}..sigma_20} span the
    one-hot basis, so host-side inversion recovers exact counts.
  - joint counts via TensorE: planes(yp)^T @ planes(y), 6 pixel-columns per
    matmul ([128, BW] x [128, 126]) accumulated into one PSUM [128, 126]
    f32 tile (exact integers < 2^24),
  - host: sum 6 diagonal 21x21 blocks (g::6), invert the basis transform,
    and histogram the (<1%) tail columns that skip the device entirely.
"""

import numpy as np


def _ensure_axon_hooks_stub():
    """bass_utils imports antenv.axon_hooks when tracing is requested (e.g.
    via BASS_TRACE=1 in the environment). Some images lack that module; a
    stub returning no hook makes bass_utils degrade gracefully (skip trace,
    still run) instead of crashing. Never shadows a real module."""
    try:
        import antenv.axon_hooks  # noqa: F401
    except ImportError:
        import sys
        import types

        try:
            import antenv
        except ImportError:
            return
        mod = types.ModuleType("antenv.axon_hooks")
        mod._hook = None
        mod.set_axon_ntff_profile_hook = lambda h: setattr(mod, "_hook", h)
        mod.get_axon_ntff_profile_hook = lambda: mod._hook
        sys.modules["antenv.axon_hooks"] = mod
        antenv.axon_hooks = mod


_ensure_axon_hooks_stub()

C = 21                  # classes
G = 6                   # pixel-column groups per matmul (G*C = 126 <= 128)
P = 128                 # partitions
BW = 128                # block width: 126 used + 2 pad (128 => FWL weight load)
FP = 972                # pixel-columns per tensor per chunk (divisible by 6)
N_ACT = 4               # top classes as sign-step planes on ScalarE

_CACHE = {}


def _build(
    n_free,
    fp=FP,
    bw=BW,
    n_act=N_ACT,
    work_cols=None,
    repeat=1,
    skip_mm=False,
    cat_bufs=3,
    plane_bufs=2,
):
    import concourse.bacc as bacc
    import concourse.mybir as mybir
    import concourse.tile as tile
    from contextlib import nullcontext

    if work_cols is None:
        work_cols = n_free

    nc = bacc.Bacc(
        "TRN2",
        target_bir_lowering=False,
        debug=False,
        enable_asserts=False,
        num_devices=8,
    )
    yp = nc.dram_tensor("yp", [P, n_free], mybir.dt.int32, kind="ExternalInput").ap()
    y = nc.dram_tensor("y", [P, n_free], mybir.dt.int32, kind="ExternalInput").ap()
    out = nc.dram_tensor("out", [P, 126], mybir.dt.float32, kind="ExternalOutput").ap()

    # Only full chunks run on-device; the remainder (< fp cols, <1% of
    # pixels) is histogrammed on the host with np.bincount. The last full
    # chunk is split small-tapered so the final matmul drain is short.
    n_main = (work_cols // fp) * fp
    chunk_ws = [fp] * (n_main // fp)
    if chunk_ws and fp >= 432:
        chunk_ws[-1:] = [fp - 216, 216]   # short final chunk: quick drain
        chunk_ws[:1] = [216, fp - 216]    # short first chunk: quick fill
    total_mms = n_main // G

    bf16 = mybir.dt.bfloat16
    f32 = mybir.dt.float32
    i32 = mybir.dt.int32
    Copy = mybir.ActivationFunctionType.Copy
    Sign = mybir.ActivationFunctionType.Sign
    n_dve = C - 1 - n_act        # ones plane sits at u = n_dve

    with tile.TileContext(nc) as tc:
        with (
            tc.tile_pool(name="psum", bufs=1, space="PSUM") as psum_pool,
            tc.tile_pool(name="cat", bufs=cat_bufs) as cat_pool,
            tc.tile_pool(name="planes", bufs=plane_bufs) as plane_pool,
            tc.tile_pool(name="singles", bufs=1) as singles,
        ):
            acc = psum_pool.tile([P, 126], f32)
            if n_act:
                bias_t = singles.tile([P, n_act], f32)
                for k in range(n_act):
                    nc.vector.memset(bias_t[:, k : k + 1], 0.5 - (n_dve + 1 + k))
                # dummy activation: trigger the ACT table load during the
                # preamble/first-DMA window instead of on the critical path
                warm = singles.tile([P, 1], bf16)
                nc.scalar.activation(warm[:], bias_t[:, 0:1], Sign)
            mm = 0
            chunk_idx = 0
            rep_ctx = tc.For_i(0, repeat, 1) if repeat > 1 else nullcontext()

            with rep_ctx:

                def do_plane_chunk(c16, wp, o, w):
                    """c16: [128, 2*wp] bf16 = [yp pair-cols | y pair-cols].
                    Process the chunk at column offset o, width w (w % 6 == 0).

                    planes[p, blk*BW + u*6 + g] = feat_u(vals[p, blk*6+g]),
                    blk in [0, 2*w/6). A-side = blks [0, w/6), B-side = rest.
                    Each matmul reads a contiguous [128, BW] slice.
                    """
                    nonlocal mm, chunk_idx
                    nblk = 2 * w // G
                    catv = (
                        c16.rearrange("p (s f) -> p s f", s=2)[:, :, o : o + w]
                        .rearrange("p s (b f) -> p s b f", f=G)
                    )
                    planes = plane_pool.tile([P, nblk_max * BW], bf16, tag="planes")
                    pl3 = planes[:, : nblk * BW].rearrange("p (b f) -> p b f", f=BW)
                    pl4 = planes[:, : nblk * BW].rearrange(
                        "p (s b f) -> p s b f", s=2, f=BW
                    )
                    if chunk_idx < plane_bufs:
                        # constant columns, written once per pool buffer: the
                        # ones plane (u = n_dve) and, for bw=128, the 2 pad
                        # columns. Pool bufs rotate round-robin so these
                        # persist; later chunks never touch them. Memset the
                        # full-size view so every chunk size is covered.
                        plf = planes[:, :].rearrange("p (b f) -> p b f", f=BW)
                        nc.vector.memset(
                            plf[:, :, n_dve * G : (n_dve + 1) * G], 1.0
                        )
                        if bw == 128:
                            nc.vector.memset(plf[:, :, 126:128], 0.0)
                    chunk_idx += 1
                    for u in range(n_dve):
                        nc.vector.tensor_scalar(
                            pl4[:, :, :, u * G : (u + 1) * G],
                            catv[:],
                            float(u),
                            None,
                            mybir.AluOpType.is_equal,
                        )
                    for u in range(n_dve + 1, C):
                        k = u - n_dve - 1
                        nc.scalar.activation(
                            pl4[:, :, :, u * G : (u + 1) * G],
                            catv[:],
                            Sign,
                            bias=bias_t[:, k : k + 1],
                        )
                    half = w // G
                    for t in (range(0) if skip_mm else range(w // G)):
                        nc.tensor.matmul(
                            acc[:, :],
                            pl3[:, t, :bw].rearrange("p f -> p f"),
                            pl3[:, half + t, :126].rearrange("p f -> p f"),
                            start=(mm == 0),
                            stop=(mm == total_mms - 1),
                        )
                        mm += 1

                nblk_max = 2 * fp // G

                off = 0
                for w in chunk_ws:
                    cat32 = cat_pool.tile([P, 2 * fp], i32, tag="cat32")
                    nc.sync.dma_start(cat32[:, :w], yp[:, off : off + w])
                    nc.sync.dma_start(
                        cat32[:, w : 2 * w], y[:, off : off + w]
                    )
                    cat16 = cat_pool.tile([P, 2 * fp], bf16, tag="cat16")
                    nc.scalar.activation(
                        cat16[:, : 2 * w], cat32[:, : 2 * w], Copy
                    )
                    do_plane_chunk(cat16[:, : 2 * w], w, 0, w)
                    off += w

            assert skip_mm or mm == total_mms
            res = singles.tile([P, 126], f32)
            if skip_mm:
                nc.vector.memset(res[:], 0.0)
            else:
                nc.vector.tensor_copy(res[:], acc[:, :])
            nc.sync.dma_start(out, res[:])

    nc.compile()
    return nc


def _feature_eval(n_act, x):
    """Feature vector phi(x): deltas, ones, sign-steps. Works for any x
    (including the sentinel)."""
    n_dve = C - 1 - n_act
    v = np.zeros(C, dtype=np.float64)
    for u in range(n_dve):
        v[u] = 1.0 if x == u else 0.0
    v[n_dve] = 1.0
    for u in range(n_dve + 1, C):
        v[u] = 1.0 if x >= u else -1.0
    return v


def _basis_matrix(n_act):
    """Phi[u, x] = feature u evaluated at class value x."""
    return np.stack([_feature_eval(n_act, x) for x in range(C)], axis=1)


def _get(n_free):
    if n_free not in _CACHE:
        _CACHE[n_free] = _build(n_free)
    return _CACHE[n_free]


def kernel(yp, y, res, n_classes, _trace=False):
    from concourse import bass_utils

    yp = np.ascontiguousarray(np.asarray(yp))
    y = np.ascontiguousarray(np.asarray(y))
    B = yp.shape[0]
    n_free = yp[0].size // P
    nc = _get(n_free)
    in_maps = [
        {"yp": yp[b].reshape(P, n_free), "y": y[b].reshape(P, n_free)}
        for b in range(B)
    ]
    r = bass_utils.run_bass_kernel_spmd(
        nc, in_maps, core_ids=list(range(B)), trace=_trace
    )

    # Host-side reconstruction
    n_act = N_ACT
    n_main = (n_free // FP) * FP
    phi = _basis_matrix(n_act)
    phi_inv = np.linalg.inv(phi)

    outs = []
    for b in range(B):
        Pm = r.results[b]["out"].astype(np.float64)
        M = np.zeros((C, C), np.float64)
        for g in range(G):
            M += Pm[g:126:G, g:126:G]
        Cb = phi_inv @ M @ phi_inv.T
        if n_main < n_free:  # tail pixels histogrammed on host
            ypt = yp[b].reshape(P, n_free)[:, n_main:].ravel()
            yt = y[b].reshape(P, n_free)[:, n_main:].ravel()
            Cb = Cb + np.bincount(
                ypt * C + yt, minlength=C * C
            ).reshape(C, C)
        outs.append(Cb)
    res_np = np.stack(outs).astype(np.float32)
    if _trace:
        kernel._last_results = r
    return res_np


# revision 33
# speedup vs baseline: 1.0531x; 1.0017x over previous
"""Confusion-matrix (joint histogram) kernel for Trainium2.

Math: out[b, i, j] = #{pixels p in batch b : yp[b,p] == i and y[b,p] == j}
for i, j in [0, 21). Inputs yp, y are [8, 2048, 2048] int32, values in [0, 21).

Per NeuronCore (core c processes batch c):
  - DMA int32 pixel chunks into SBUF; ScalarE converts to bf16,
  - feature planes in matmul-ready interleaved layout
    (planes[p, blk*BW + u*6 + g]), split across engines by feature kind:
      u < D:  delta planes  (x == u)        via DVE tensor_scalar(is_equal), 4x
      u == D: constant ones plane           via one-time memset (pool bufs
              are reused round-robin, so the ones/pad columns persist)
      u > D:  sign-step planes sign(x-u+.5) via ScalarE activation(Sign), +-1
    The 21 features {delta_0..delta_{D-1}, 1, sigma_{D+1}..sigma_20} span the
    one-hot basis, so host-side inversion recovers exact counts.
  - joint counts via TensorE: planes(yp)^T @ planes(y), 6 pixel-columns per
    matmul ([128, BW] x [128, 126]) accumulated into one PSUM [128, 126]
    f32 tile (exact integers < 2^24),
  - host: sum 6 diagonal 21x21 blocks (g::6), invert the basis transform,
    and histogram the (<1%) tail columns that skip the device entirely.
"""

import numpy as np


def _ensure_axon_hooks_stub():
    """bass_utils imports antenv.axon_hooks when tracing is requested (e.g.
    via BASS_TRACE=1 in the environment). Some images lack that module; a
    stub returning no hook makes bass_utils degrade gracefully (skip trace,
    still run) instead of crashing. Never shadows a real module."""
    try:
        import antenv.axon_hooks  # noqa: F401
    except ImportError:
        import sys
        import types

        try:
            import antenv
        except ImportError:
            return
        mod = types.ModuleType("antenv.axon_hooks")
        mod._hook = None
        mod.set_axon_ntff_profile_hook = lambda h: setattr(mod, "_hook", h)
        mod.get_axon_ntff_profile_hook = lambda: mod._hook
        sys.modules["antenv.axon_hooks"] = mod
        antenv.axon_hooks = mod


_ensure_axon_hooks_stub()

C = 21                  # classes
G = 6                   # pixel-column groups per matmul (G*C = 126 <= 128)
P = 128                 # partitions
BW = 128                # block width: 126 used + 2 pad (128 => FWL weight load)
FP = 972                # pixel-columns per tensor per chunk (divisible by 6)
N_ACT = 4               # top classes as sign-step planes on ScalarE

_CACHE = {}


def _build(
    n_free,
    fp=FP,
    bw=BW,
    n_act=N_ACT,
    work_cols=None,
    repeat=1,
    skip_mm=False,
    cat_bufs=3,
    plane_bufs=2,
):
    import concourse.bacc as bacc
    import concourse.mybir as mybir
    import concourse.tile as tile
    from contextlib import nullcontext

    if work_cols is None:
        work_cols = n_free

    nc = bacc.Bacc(
        "TRN2",
        target_bir_lowering=False,
        debug=False,
        enable_asserts=False,
        num_devices=8,
    )
    yp = nc.dram_tensor("yp", [P, n_free], mybir.dt.int32, kind="ExternalInput").ap()
    y = nc.dram_tensor("y", [P, n_free], mybir.dt.int32, kind="ExternalInput").ap()
    out = nc.dram_tensor("out", [P, 126], mybir.dt.float32, kind="ExternalOutput").ap()

    # Only full chunks run on-device; the remainder (< fp cols, <1% of
    # pixels) is histogrammed on the host with np.bincount. The last full
    # chunk is split small-tapered so the final matmul drain is short.
    n_main = (work_cols // fp) * fp
    chunk_ws = [fp] * (n_main // fp)
    if chunk_ws and fp >= 432:
        chunk_ws[-1:] = [fp - 216, 216]   # short final chunk: quick drain
        chunk_ws[:1] = [216, fp - 216]    # short first chunk: quick fill
    total_mms = n_main // G

    bf16 = mybir.dt.bfloat16
    f32 = mybir.dt.float32
    i32 = mybir.dt.int32
    Copy = mybir.ActivationFunctionType.Copy
    Sign = mybir.ActivationFunctionType.Sign
    n_dve = C - 1 - n_act        # ones plane sits at u = n_dve

    with tile.TileContext(nc) as tc:
        with (
            tc.tile_pool(name="psum", bufs=1, space="PSUM") as psum_pool,
            tc.tile_pool(name="cat", bufs=cat_bufs) as cat_pool,
            tc.tile_pool(name="planes", bufs=plane_bufs) as plane_pool,
            tc.tile_pool(name="singles", bufs=1) as singles,
        ):
            acc = psum_pool.tile([P, 126], f32)
            if n_act:
                bias_t = singles.tile([P, n_act], f32)
                for k in range(n_act):
                    nc.vector.memset(bias_t[:, k : k + 1], 0.5 - (n_dve + 1 + k))
                # dummy activation: trigger the ACT table load during the
                # preamble/first-DMA window instead of on the critical path
                warm = singles.tile([P, 1], bf16)
                nc.scalar.activation(warm[:], bias_t[:, 0:1], Sign)
            mm = 0
            chunk_idx = 0
            rep_ctx = tc.For_i(0, repeat, 1) if repeat > 1 else nullcontext()

            with rep_ctx:

                def do_plane_chunk(c16, wp, o, w):
                    """c16: [128, 2*wp] bf16 = [yp pair-cols | y pair-cols].
                    Process the chunk at column offset o, width w (w % 6 == 0).

                    planes[p, blk*BW + u*6 + g] = feat_u(vals[p, blk*6+g]),
                    blk in [0, 2*w/6). A-side = blks [0, w/6), B-side = rest.
                    Each matmul reads a contiguous [128, BW] slice.
                    """
                    nonlocal mm, chunk_idx
                    nblk = 2 * w // G
                    catv = (
                        c16.rearrange("p (s f) -> p s f", s=2)[:, :, o : o + w]
                        .rearrange("p s (b f) -> p s b f", f=G)
                    )
                    planes = plane_pool.tile([P, nblk_max * BW], bf16, tag="planes")
                    pl3 = planes[:, : nblk * BW].rearrange("p (b f) -> p b f", f=BW)
                    pl4 = planes[:, : nblk * BW].rearrange(
                        "p (s b f) -> p s b f", s=2, f=BW
                    )
                    if chunk_idx < plane_bufs:
                        # constant columns, written once per pool buffer: the
                        # ones plane (u = n_dve) and, for bw=128, the 2 pad
                        # columns. Pool bufs rotate round-robin so these
                        # persist; later chunks never touch them. Memset the
                        # full-size view so every chunk size is covered.
                        plf = planes[:, :].rearrange("p (b f) -> p b f", f=BW)
                        nc.vector.memset(
                            plf[:, :, n_dve * G : (n_dve + 1) * G], 1.0
                        )
                        if bw == 128:
                            nc.vector.memset(plf[:, :, 126:128], 0.0)
                    chunk_idx += 1
                    for u in range(n_dve):
                        nc.vector.tensor_scalar(
                            pl4[:, :, :, u * G : (u + 1) * G],
                            catv[:],
                            float(u),
                            None,
                            mybir.AluOpType.is_equal,
                        )
                    for u in range(n_dve + 1, C):
                        k = u - n_dve - 1
                        nc.scalar.activation(
                            pl4[:, :, :, u * G : (u + 1) * G],
                            catv[:],
                            Sign,
                            bias=bias_t[:, k : k + 1],
                        )
                    half = w // G
                    for t in (range(0) if skip_mm else range(w // G)):
                        nc.tensor.matmul(
                            acc[:, :],
                            pl3[:, t, :bw].rearrange("p f -> p f"),
                            pl3[:, half + t, :126].rearrange("p f -> p f"),
                            start=(mm == 0),
                            stop=(mm == total_mms - 1),
                        )
                        mm += 1

                nblk_max = 2 * fp // G

                off = 0
                for w in chunk_ws:
                    cat32 = cat_pool.tile([P, 2 * fp], i32, tag="cat32")
                    nc.sync.dma_start(cat32[:, :w], yp[:, off : off + w])
                    nc.sync.dma_start(
                        cat32[:, w : 2 * w], y[:, off : off + w]
                    )
                    cat16 = cat_pool.tile([P, 2 * fp], bf16, tag="cat16")
                    nc.scalar.activation(
                        cat16[:, : 2 * w], cat32[:, : 2 * w], Copy
                    )
                    do_plane_chunk(cat16[:, : 2 * w], w, 0, w)
                    off += w

            assert skip_mm or mm == total_mms
            res = singles.tile([P, 126], f32)
            if skip_mm:
                nc.vector.memset(res[:], 0.0)
            else:
                nc.vector.tensor_copy(res[:], acc[:, :])
            nc.sync.dma_start(out, res[:])

    nc.compile()
    return nc


def _feature_eval(n_act, x):
    """Feature vector phi(x): deltas, ones, sign-steps. Works for any x
    (including the sentinel)."""
    n_dve = C - 1 - n_act
    v = np.zeros(C, dtype=np.float64)
    for u in range(n_dve):
        v[u] = 1.0 if x == u else 0.0
    v[n_dve] = 1.0
    for u in range(n_dve + 1, C):
        v[u] = 1.0 if x >= u else -1.0
    return v


def _basis_matrix(n_act):
    """Phi[u, x] = feature u evaluated at class value x."""
    return np.stack([_feature_eval(n_act, x) for x in range(C)], axis=1)


def _get(n_free):
    if n_free not in _CACHE:
        _CACHE[n_free] = _build(n_free)
    return _CACHE[n_free]


def kernel(yp, y, res, n_classes, _trace=False):
    from concourse import bass_utils

    yp = np.ascontiguousarray(np.asarray(yp))
    y = np.ascontiguousarray(np.asarray(y))
    B = yp.shape[0]
    n_free = yp[0].size // P
    nc = _get(n_free)
    in_maps = [
        {"yp": yp[b].reshape(P, n_free), "y": y[b].reshape(P, n_free)}
        for b in range(B)
    ]
    r = bass_utils.run_bass_kernel_spmd(
        nc, in_maps, core_ids=list(range(B)), trace=_trace
    )

    # Host-side reconstruction
    n_act = N_ACT
    n_main = (n_free // FP) * FP
    phi = _basis_matrix(n_act)
    phi_inv = np.linalg.inv(phi)

    outs = []
    for b in range(B):
        Pm = r.results[b]["out"].astype(np.float64)
        M = np.zeros((C, C), np.float64)
        for g in range(G):
            M += Pm[g:126:G, g:126:G]
        Cb = phi_inv @ M @ phi_inv.T
        if n_main < n_free:  # tail pixels histogrammed on host
            ypt = yp[b].reshape(P, n_free)[:, n_main:].ravel()
            yt = y[b].reshape(P, n_free)[:, n_main:].ravel()
            Cb = Cb + np.bincount(
                ypt * C + yt, minlength=C * C
            ).reshape(C, C)
        outs.append(Cb)
    res_np = np.stack(outs).astype(np.float32)
    if _trace:
        kernel._last_results = r
    return res_np
